# revision 1
# baseline (speedup 1.0000x reference)
"""Multi-head attention kernel for Trainium2, 8 NeuronCores.

Problem: B=4, T=2048, D=1024, 16 heads x 64 head-dim, key-padding mask.
Sharding: core = (batch b, head-half hh); each of the 8 cores computes one
batch's attention over 8 heads (512 channels) and a partial output
projection; the host sums the two partials per batch and adds the bias.

Device-side layout is channel-major throughout:
  hT   = hidden[b]^T                      [D, T]    (bf16)
  Q^T  = (Wq_half)^T hT                   [C, T]    via lhsT=Wq, rhs=hT
  K^T/V computed only at unmasked key positions (host compacts keys using
  the attention mask, zero-padded to TKV).
  S^T  = K^T-block^T Q^T                  [t2, t1]  PSUM; the two heads of a
         pair are row-tiled (K=64 at partition bases 0/64) so their matmuls
         overlap in the PE array.
  P^T  = exp(scale*S^T + bias[t2])        (ACT, bias=-1e9 at padded keys)
  O    pair-accumulated col-tiled: head A -> PSUM rows 0:64, head B -> rows
       64:128 of one bank; a parallel rowsum bank accumulates ones^T P^T
       replicated across 64 rows per head => denominator aligned with O.
  out  = (O/denom) concat @ Wo_half       [T, D]    (partial, fp32)
"""

import numpy as np
import ml_dtypes

import concourse.bacc as bacc
import concourse.tile as tile
from concourse.tile import add_dep_helper
from concourse import mybir
from concourse.bass_utils import run_bass_kernel_spmd

BF16 = mybir.dt.bfloat16
F32 = mybir.dt.float32

B, T, D = 4, 2048, 1024
HEADS, DH = 16, 64
C = 512              # channels per core (8 heads)
SCALE = DH ** -0.5
NEG = -1.0e9

TKV = 1152           # compacted+padded key count (seed-0 max unmasked is 1075)
KB = D // 128        # 8 contraction blocks for the projections


def build_bass(tkv=TKV):
    nblk = tkv // 128
    nc = bacc.Bacc("TRN2", target_bir_lowering=False)

    hT_d = nc.dram_tensor("hT", [D, T], BF16, kind="ExternalInput")
    hTkv_d = nc.dram_tensor("hTkv", [D, tkv], BF16, kind="ExternalInput")
    wq_d = nc.dram_tensor("wq", [D, C], BF16, kind="ExternalInput")
    wk_d = nc.dram_tensor("wk", [D, C], BF16, kind="ExternalInput")
    wv_d = nc.dram_tensor("wv", [D, C], BF16, kind="ExternalInput")
    wo_d = nc.dram_tensor("wo", [C, D], BF16, kind="ExternalInput")
    mb_d = nc.dram_tensor("mb", [128, nblk], F32, kind="ExternalInput")
    out_d = nc.dram_tensor("out", [T, D], F32, kind="ExternalOutput")

    with tile.TileContext(nc) as tc:
        with (
            tc.tile_pool(name="persist", bufs=1) as persist,
            tc.tile_pool(name="pt", bufs=4) as ppool,
            tc.tile_pool(name="rs", bufs=4) as rpool,
            tc.tile_pool(name="ostage", bufs=4) as ostage,
        ):
            # ---- persistent SBUF tensors + input DMA ----
            hT = persist.tile([128, KB, T], BF16)
            hTkv = persist.tile([128, KB, tkv], BF16)
            wq = persist.tile([128, KB, C], BF16)
            wk = persist.tile([128, KB, C], BF16)
            wv = persist.tile([128, KB, C], BF16)
            wo = persist.tile([128, C // 128, D], BF16)
            mb = persist.tile([128, nblk], F32)
            qT = persist.tile([128, C // 128, T], BF16)
            kT = persist.tile([128, C // 128, tkv], BF16)
            vsb = persist.tile([128, nblk, C], BF16)
            ones64 = persist.tile([128, 64], BF16)
            ocatT = persist.tile([128, C // 128, T], BF16)

            hTkv_r = hTkv_d.ap().rearrange("(k p) t -> p k t", p=128)
            hT_r = hT_d.ap().rearrange("(k p) t -> p k t", p=128)
            wv_r = wv_d.ap().rearrange("(k p) c -> p k c", p=128)
            wk_r = wk_d.ap().rearrange("(k p) c -> p k c", p=128)
            wq_r = wq_d.ap().rearrange("(k p) c -> p k c", p=128)
            nc.gpsimd.dma_start(wv[:], wv_r)
            _engs = [nc.sync, nc.gpsimd]
            for k in range(KB):
                _engs[k % 2].dma_start(hTkv[:, k, :], hTkv_r[:, k, :])
            nc.sync.dma_start(wk[:], wk_r)
            nc.sync.dma_start(wq[:], wq_r)
            nc.sync.dma_start(mb[:], mb_d.ap())
            for k in range(KB):
                eng = nc.gpsimd if k % 2 == 0 else nc.sync
                eng.dma_start(hT[:, k, :], hT_r[:, k, :])
            nc.gpsimd.dma_start(wo[:], wo_d.ap().rearrange("(k p) e -> p k e", p=128))
            nc.vector.memset(ones64[:], 1.0)

            # ---- phase 1: QKV projections ----
            # V first, then K/Q per head-pair, so attention (pair 0) can begin
            # while later pairs' projections still run.
            with tc.tile_pool(name="qkv_ps", bufs=4, space="PSUM") as qps:
                def emit_v(tb):
                    ps = qps.tile([128, 512], F32, tag="qkv", name=f"vps_{tb}")
                    for k in range(KB):
                        nc.tensor.matmul(
                            ps[:],
                            hTkv[:, k, tb * 128:(tb + 1) * 128],
                            wv[:, k, :],
                            start=(k == 0), stop=(k == KB - 1),
                        )
                    nc.vector.tensor_copy(vsb[:, tb, :], ps[:])

                def emit_k(cb):
                    for t0 in range(0, tkv, 512):
                        w = min(512, tkv - t0)
                        ps = qps.tile([128, 512], F32, tag="qkv", name=f"kps_{cb}_{t0}")
                        for k in range(KB):
                            nc.tensor.matmul(
                                ps[:, :w],
                                wk[:, k, cb * 128:(cb + 1) * 128],
                                hTkv[:, k, t0:t0 + w],
                                start=(k == 0), stop=(k == KB - 1),
                            )
                        nc.vector.tensor_copy(kT[:, cb, t0:t0 + w], ps[:, :w])

                def emit_q(cb):
                    for tch in range(T // 512):
                        ps = qps.tile([128, 512], F32, tag="qkv", name=f"qps_{cb}_{tch}")
                        for k in range(KB):
                            nc.tensor.matmul(
                                ps[:],
                                wq[:, k, cb * 128:(cb + 1) * 128],
                                hT[:, k, tch * 512:(tch + 1) * 512],
                                start=(k == 0), stop=(k == KB - 1),
                            )
                        nc.vector.tensor_copy(qT[:, cb, tch * 512:(tch + 1) * 512], ps[:])

                for tb in range(nblk):
                    emit_v(tb)
                for cb in range(C // 128):
                    emit_k(cb)
                    emit_q(cb)

            # ---- phase 2: attention (t1-half outer, head pair inner) ----
            with (
                tc.tile_pool(name="s_ps", bufs=2, space="PSUM") as spool,
                tc.tile_pool(name="o_ps", bufs=2, space="PSUM") as opool,
                tc.tile_pool(name="r_ps", bufs=2, space="PSUM") as rps,
                tc.tile_pool(name="osb", bufs=4) as osbp,
            ):
                def emit_proj(tts, pools=None):
                    pools = pools or [(spool, "s")]
                    for tt in tts:
                        ot = ostage.tile([128, 1024], F32, tag="os", name=f"ot_{tt}")
                        for e in range(D // 512):
                            pool_i, tag_i = pools[(2 * tt + e) % len(pools)]
                            ps = pool_i.tile([128, 512], F32, tag=tag_i, name=f"pj_{tt}_{e}")
                            for cbj in range(C // 128):
                                nc.tensor.matmul(
                                    ps[:],
                                    ocatT[:, cbj, tt * 128:(tt + 1) * 128],
                                    wo[:, cbj, e * 512:(e + 1) * 512],
                                    start=(cbj == 0), stop=(cbj == C // 128 - 1),
                                )
                            nc.vector.tensor_copy(ot[:, e * 512:(e + 1) * 512], ps[:])
                        nc.sync.dma_start(
                            out_d.ap()[tt * 128:(tt + 1) * 128, :], ot[:],
                        )

                for half in range(2):
                    t1o = half * 1024
                    for cb in range(C // 128):
                        hA, hB = 2 * cb, 2 * cb + 1
                        ops = [opool.tile([128, 512], F32, tag="o", name=f"o_{cb}_{half}_{i}")
                               for i in range(2)]
                        rss = [rps.tile([128, 512], F32, tag="r", name=f"r_{cb}_{half}_{i}")
                               for i in range(2)]
                        for blk in range(nblk):
                            sa = spool.tile([128, 1024], F32, tag="s")
                            sb_t = spool.tile([128, 1024], F32, tag="s")
                            for c2 in range(2):
                                t1s = slice(t1o + c2 * 512, t1o + (c2 + 1) * 512)
                                nc.tensor.matmul(
                                    sa[:, c2 * 512:(c2 + 1) * 512],
                                    kT[0:64, cb, blk * 128:(blk + 1) * 128],
                                    qT[0:64, cb, t1s],
                                    start=True, stop=True,
                                )
                                nc.tensor.matmul(
                                    sb_t[:, c2 * 512:(c2 + 1) * 512],
                                    kT[64:128, cb, blk * 128:(blk + 1) * 128],
                                    qT[64:128, cb, t1s],
                                    start=True, stop=True,
                                )
                            pta = ppool.tile([128, 1024], BF16, tag="pt")
                            nc.scalar.activation(
                                pta[:], sa[:], mybir.ActivationFunctionType.Exp,
                                bias=mb[:, blk:blk + 1], scale=SCALE,
                            )
                            ptb = ppool.tile([128, 1024], BF16, tag="pt")
                            exp_b_inst = nc.scalar.activation(
                                ptb[:], sb_t[:], mybir.ActivationFunctionType.Exp,
                                bias=mb[:, blk:blk + 1], scale=SCALE,
                            )
                            st, sp = (blk == 0), (blk == nblk - 1)
                            for c2 in range(2):
                                c2s = slice(c2 * 512, (c2 + 1) * 512)
                                # col-tiled pair: head A -> rows 0:64, head B -> rows 64:128
                                # of one bank; pending-zero is per-partition, so each half
                                # starts its own accumulation group at blk 0.
                                mm_a = nc.tensor.matmul(
                                    ops[c2][0:64, :], vsb[:, blk, hA * 64:hA * 64 + 64],
                                    pta[:, c2s], start=st, stop=sp,
                                    tile_position=(0, 0), skip_group_check=True,
                                )
                                if c2 == 0:
                                    add_dep_helper(exp_b_inst.ins, mm_a.ins, sync=True,
                                                   reason="pair-issue O_A with O_B")
                                nc.tensor.matmul(
                                    ops[c2][64:128, :], vsb[:, blk, hB * 64:hB * 64 + 64],
                                    ptb[:, c2s], start=st, stop=sp,
                                    tile_position=(0, 64), skip_group_check=True,
                                )
                                nc.tensor.matmul(
                                    rss[c2][0:64, :], ones64[:],
                                    pta[:, c2s], start=st, stop=sp,
                                    tile_position=(0, 0), skip_group_check=True,
                                )
                                nc.tensor.matmul(
                                    rss[c2][64:128, :], ones64[:],
                                    ptb[:, c2s], start=st, stop=sp,
                                    tile_position=(0, 64), skip_group_check=True,
                                )
                        # evict PSUM fast (frees banks for the next pair), then
                        # normalize from SBUF off the critical path
                        for c2 in range(2):
                            t1s = slice(t1o + c2 * 512, t1o + (c2 + 1) * 512)
                            o_sb = osbp.tile([128, 512], F32, tag="ob")
                            nc.vector.tensor_copy(o_sb[:], ops[c2][:])
                            rsb = rpool.tile([128, 512], F32, tag="rs")
                            nc.vector.reciprocal_approx_fast(rsb[:], rss[c2][:])
                            nc.vector.tensor_mul(ocatT[:, cb, t1s], o_sb[:], rsb[:])
                    if half == 0:
                        emit_proj(range(0, 8),
                                  pools=[(spool, "s"), (opool, "o"), (rps, "r")])
                    else:
                        emit_proj(range(8, 16),
                                  pools=[(spool, "s"), (opool, "o"), (rps, "r")])

    nc.compile()
    return nc


_NC_CACHE = {}


def _get_nc(tkv=TKV):
    if tkv not in _NC_CACHE:
        _NC_CACHE[tkv] = build_bass(tkv)
    return _NC_CACHE[tkv]


def make_in_maps(inputs, tkv=TKV):
    hidden = np.asarray(inputs["hidden_states"], np.float32)
    mask = np.asarray(inputs["attention_mask"])
    Wq = np.asarray(inputs["Wq"], np.float32).astype(ml_dtypes.bfloat16)
    Wk = np.asarray(inputs["Wk"], np.float32).astype(ml_dtypes.bfloat16)
    Wv = np.asarray(inputs["Wv"], np.float32).astype(ml_dtypes.bfloat16)
    Wo = np.asarray(inputs["Wo"], np.float32).astype(ml_dtypes.bfloat16)

    nblk = tkv // 128
    in_maps = []
    for core in range(8):
        b, hh = divmod(core, 2)
        ch = slice(hh * C, (hh + 1) * C)
        hTb = np.ascontiguousarray(hidden[b].T).astype(ml_dtypes.bfloat16)
        idx = np.nonzero(mask[b])[0]
        n = len(idx)
        assert n <= tkv, f"unmasked keys {n} > TKV {tkv}"
        hTkv = np.zeros((D, tkv), ml_dtypes.bfloat16)
        hTkv[:, :n] = hidden[b].T[:, idx].astype(ml_dtypes.bfloat16)
        mbv = np.full(tkv, NEG, np.float32)
        mbv[:n] = 0.0
        mbv = np.ascontiguousarray(mbv.reshape(nblk, 128).T)  # [128, nblk]
        in_maps.append({
            "hT": hTb,
            "hTkv": hTkv,
            "wq": np.ascontiguousarray(Wq[:, ch]),
            "wk": np.ascontiguousarray(Wk[:, ch]),
            "wv": np.ascontiguousarray(Wv[:, ch]),
            "wo": np.ascontiguousarray(Wo[ch, :]),
            "mb": mbv,
        })
    return in_maps


def gather(results, inputs):
    bo = np.asarray(inputs["bo"], np.float32)
    out = np.empty((B, T, D), np.float32)
    for b in range(B):
        out[b] = results[2 * b]["out"] + results[2 * b + 1]["out"] + bo
    return out


def _pick_tkv(inputs):
    mask = np.asarray(inputs["attention_mask"])
    nmax = int(mask.sum(axis=1).max())
    return max(TKV, -(-nmax // 128) * 128)


def _run(inputs, trace=False):
    tkv = _pick_tkv(inputs)
    nc = _get_nc(tkv)
    in_maps = make_in_maps(inputs, tkv)
    res = run_bass_kernel_spmd(nc, in_maps, core_ids=list(range(8)), trace=trace)
    return gather(res.results, inputs), res


def kernel(**inputs):
    out, _ = _run(inputs)
    return out



# revision 9
# speedup vs baseline: 1.0557x; 1.0557x over previous
"""Multi-head attention kernel for Trainium2, 8 NeuronCores.

Problem: B=4, T=2048, D=1024, 16 heads x 64 head-dim, key-padding mask.
Sharding: core = (batch b, head-half hh); each of the 8 cores computes one
batch's attention over 8 heads (512 channels) and a partial output
projection; the host sums the two partials per batch and adds the bias.

Device-side layout is channel-major throughout:
  hT   = hidden[b]^T                      [D, T]    (bf16)
  Q^T  = (Wq_half)^T hT                   [C, T]    via lhsT=Wq, rhs=hT
  K^T/V computed only at unmasked key positions (host compacts keys using
  the attention mask, zero-padded to TKV).
  S^T  = K^T-block^T Q^T                  [128 keys, 1024] PSUM; both heads
         of a pair side by side (A cols 0:512, B cols 512:1024) for one
         512-query chunk, so a single ACT exp covers the pair.
  P^T  = exp(scale*S^T + bias[k])         (ACT, bias=-1e9 at padded keys)
  O    = [V_h | ones]^T P_h^T             M=65 matmul: rows 0:64 = head
         output, row 64 = softmax denominator (no separate rowsum matmuls).
  norm: PE replicate-matmul broadcasts the denominator row to 64
         partitions, DVE reciprocal_approx_fast, Pool multiply -> ocatT.
  out  = ocatT^T @ Wo_half                [T, D]    (partial, fp32)
"""

import numpy as np
import ml_dtypes

import concourse.bacc as bacc
import concourse.tile as tile
from concourse import mybir
from concourse.bass_utils import run_bass_kernel_spmd

BF16 = mybir.dt.bfloat16
F32 = mybir.dt.float32

B, T, D = 4, 2048, 1024
HEADS, DH = 16, 64
C = 512              # channels per core (8 heads)
SCALE = DH ** -0.5
NEG = -1.0e9

TKV = 1152           # compacted+padded key count (seed-0 max unmasked is 1075)
KB = D // 128        # 8 contraction blocks for the projections
NCB = C // 128       # 4 head-pair blocks
NQC = T // 512       # 4 query chunks


def build_bass(tkv=TKV):
    nblk = tkv // 128
    nc = bacc.Bacc("TRN2", target_bir_lowering=False)

    hT_d = nc.dram_tensor("hT", [D, T], BF16, kind="ExternalInput")
    hTkv_d = nc.dram_tensor("hTkv", [D, tkv], BF16, kind="ExternalInput")
    wq_d = nc.dram_tensor("wq", [D, C], BF16, kind="ExternalInput")
    wk_d = nc.dram_tensor("wk", [D, C], BF16, kind="ExternalInput")
    wv_d = nc.dram_tensor("wv", [D, C], BF16, kind="ExternalInput")
    wo_d = nc.dram_tensor("wo", [C, D], BF16, kind="ExternalInput")
    mb_d = nc.dram_tensor("mb", [128, nblk], F32, kind="ExternalInput")
    out_d = nc.dram_tensor("out", [T, D], F32, kind="ExternalOutput")

    with tile.TileContext(nc) as tc:
        with (
            tc.tile_pool(name="persist", bufs=1) as persist,
            tc.tile_pool(name="pt", bufs=3) as ppool,
            tc.tile_pool(name="osb", bufs=4) as osbp,
            tc.tile_pool(name="rsbp", bufs=4) as rsbp,
            tc.tile_pool(name="ostage", bufs=4) as ostage,
        ):
            # ---- persistent SBUF tensors ----
            hT = persist.tile([128, KB, T], BF16)
            hTkv = persist.tile([128, KB, tkv], BF16)
            wq = persist.tile([128, KB, C], BF16)
            wk = persist.tile([128, KB, C], BF16)
            wv = persist.tile([128, KB, C], BF16)
            wo = persist.tile([128, NCB, D], BF16)
            mb = persist.tile([128, nblk], F32)
            qT = persist.tile([128, NCB, T], BF16)
            kT = persist.tile([128, NCB, tkv], BF16)
            vext = persist.tile([128, nblk, 8, 65], BF16)
            ocatT = persist.tile([128, NCB, T], BF16)
            ones32 = persist.tile([128, 64], F32)
            warm = persist.tile([128, 8], F32)

            # constants first: no DMA deps, lets the exp table preload run
            # at t=0 (first Exp ACT triggers the ~2.7us table-set load)
            nc.vector.memset(ones32[:], 1.0)
            nc.vector.memset(vext[:, :, :, 64], 1.0)
            nc.scalar.activation(
                warm[:], ones32[:, 0:8], mybir.ActivationFunctionType.Exp,
            )

            # ---- input DMA, in dependency-priority order ----
            hTkv_r = hTkv_d.ap().rearrange("(k p) t -> p k t", p=128)
            hT_r = hT_d.ap().rearrange("(k p) t -> p k t", p=128)
            nc.sync.dma_start(wk[:], wk_d.ap().rearrange("(k p) c -> p k c", p=128))
            for k in range(KB):
                nc.sync.dma_start(hTkv[:, k, :], hTkv_r[:, k, :])
            nc.sync.dma_start(wq[:], wq_d.ap().rearrange("(k p) c -> p k c", p=128))
            # queries chunk-major so Q(cb=0, qc=0/1) unblocks early
            for tch in range(NQC):
                for k in range(KB):
                    nc.sync.dma_start(
                        hT[:, k, tch * 512:(tch + 1) * 512],
                        hT_r[:, k, tch * 512:(tch + 1) * 512],
                    )
            nc.gpsimd.dma_start(wv[:], wv_d.ap().rearrange("(k p) c -> p k c", p=128))
            nc.gpsimd.dma_start(mb[:], mb_d.ap())
            nc.gpsimd.dma_start(wo[:], wo_d.ap().rearrange("(k p) e -> p k e", p=128))

            # ---- projections ----
            with tc.tile_pool(name="qkv_ps", bufs=2, space="PSUM") as qps:
                def emit_v(tb):
                    ps = qps.tile([128, 512], F32, tag="qkv", name=f"vps_{tb}")
                    for k in range(KB):
                        nc.tensor.matmul(
                            ps[:],
                            hTkv[:, k, tb * 128:(tb + 1) * 128],
                            wv[:, k, :],
                            start=(k == 0), stop=(k == KB - 1),
                        )
                    # scatter the 8 heads' 64-wide slices into vext (ones col
                    # at 64 stays from the memset)
                    nc.vector.tensor_copy(vext[:, tb, :, 0:64], ps[:])

                def emit_k(cb):
                    for t0 in range(0, tkv, 512):
                        w = min(512, tkv - t0)
                        ps = qps.tile([128, 512], F32, tag="qkv", name=f"kps_{cb}_{t0}")
                        for k in range(KB):
                            nc.tensor.matmul(
                                ps[:, :w],
                                wk[:, k, cb * 128:(cb + 1) * 128],
                                hTkv[:, k, t0:t0 + w],
                                start=(k == 0), stop=(k == KB - 1),
                            )
                        nc.vector.tensor_copy(kT[:, cb, t0:t0 + w], ps[:, :w])

                def emit_q(cb, tchs):
                    for tch in tchs:
                        ps = qps.tile([128, 512], F32, tag="qkv", name=f"qps_{cb}_{tch}")
                        for k in range(KB):
                            nc.tensor.matmul(
                                ps[:],
                                wq[:, k, cb * 128:(cb + 1) * 128],
                                hT[:, k, tch * 512:(tch + 1) * 512],
                                start=(k == 0), stop=(k == KB - 1),
                            )
                        nc.vector.tensor_copy(qT[:, cb, tch * 512:(tch + 1) * 512], ps[:])

                # ---- attention ----
                with (
                    tc.tile_pool(name="s_ps", bufs=2, space="PSUM") as spool,
                    tc.tile_pool(name="o_ps", bufs=2, space="PSUM") as opool,
                ):
                    def emit_attn(cb, qc):
                        hA, hB = 2 * cb, 2 * cb + 1
                        qs = slice(qc * 512, (qc + 1) * 512)
                        oA = opool.tile([128, 512], F32, tag="o", name=f"oA_{cb}_{qc}")
                        oB = opool.tile([128, 512], F32, tag="o", name=f"oB_{cb}_{qc}")
                        for blk in range(nblk):
                            st = spool.tile([128, 1024], F32, tag="s",
                                            name=f"st_{cb}_{qc}_{blk}")
                            ks = slice(blk * 128, (blk + 1) * 128)
                            nc.tensor.matmul(
                                st[:, 0:512], kT[0:64, cb, ks], qT[0:64, cb, qs],
                                start=True, stop=True,
                            )
                            nc.tensor.matmul(
                                st[:, 512:1024], kT[64:128, cb, ks], qT[64:128, cb, qs],
                                start=True, stop=True,
                            )
                            pt = ppool.tile([128, 1024], BF16, tag="pt")
                            nc.scalar.activation(
                                pt[:], st[:], mybir.ActivationFunctionType.Exp,
                                bias=mb[:, blk:blk + 1], scale=SCALE,
                            )
                            st_, sp_ = (blk == 0), (blk == nblk - 1)
                            nc.tensor.matmul(
                                oA[0:65, :], vext[:, blk, hA, :], pt[:, 0:512],
                                start=st_, stop=sp_,
                            )
                            nc.tensor.matmul(
                                oB[0:65, :], vext[:, blk, hB, :], pt[:, 512:1024],
                                start=st_, stop=sp_,
                            )
                        # normalize: row 64 holds the denominator
                        for h, op in ((0, oA), (1, oB)):
                            o_sb = osbp.tile([65, 512], F32, tag="ob",
                                             name=f"osb_{cb}_{qc}_{h}")
                            nc.vector.tensor_copy(o_sb[:], op[0:65, :])
                            repl = opool.tile([64, 512], F32, tag="o",
                                              name=f"rp_{cb}_{qc}_{h}")
                            nc.tensor.matmul(
                                repl[:], ones32[64:65, :], o_sb[64:65, :],
                                start=True, stop=True,
                            )
                            rsb = rsbp.tile([64, 512], F32, tag="rs",
                                            name=f"rsb_{cb}_{qc}_{h}")
                            nc.vector.reciprocal_approx_fast(rsb[:], repl[:])
                            rows = slice(h * 64, (h + 1) * 64)
                            nc.gpsimd.tensor_mul(
                                ocatT[rows, cb, qs], o_sb[0:64, :], rsb[:],
                            )

                    def emit_proj(qc):
                        for tt in range(4 * qc, 4 * qc + 4):
                            ot = ostage.tile([128, 1024], F32, tag="os",
                                             name=f"ot_{tt}")
                            for e in range(2):
                                ps = opool.tile([128, 512], F32, tag="o",
                                                name=f"pj_{tt}_{e}")
                                for cbj in range(NCB):
                                    nc.tensor.matmul(
                                        ps[:],
                                        ocatT[:, cbj, tt * 128:(tt + 1) * 128],
                                        wo[:, cbj, e * 512:(e + 1) * 512],
                                        start=(cbj == 0), stop=(cbj == NCB - 1),
                                    )
                                nc.vector.tensor_copy(
                                    ot[:, e * 512:(e + 1) * 512], ps[:])
                            nc.sync.dma_start(
                                out_d.ap()[tt * 128:(tt + 1) * 128, :], ot[:],
                            )

                    # emission order interleaves projections with attention so
                    # the PE has fill work while ACT streams exps
                    for tb in range(nblk):
                        emit_v(tb)
                    emit_k(0)
                    emit_q(0, range(NQC))
                    for qc in range(NQC):
                        for cb in range(NCB):
                            if qc == 0 and cb + 1 < NCB:
                                emit_k(cb + 1)
                                emit_q(cb + 1, range(NQC))
                            emit_attn(cb, qc)
                        emit_proj(qc)

    nc.compile()
    return nc


_NC_CACHE = {}


def _get_nc(tkv=TKV):
    if tkv not in _NC_CACHE:
        _NC_CACHE[tkv] = build_bass(tkv)
    return _NC_CACHE[tkv]


def make_in_maps(inputs, tkv=TKV):
    hidden = np.asarray(inputs["hidden_states"], np.float32)
    mask = np.asarray(inputs["attention_mask"])
    Wq = np.asarray(inputs["Wq"], np.float32).astype(ml_dtypes.bfloat16)
    Wk = np.asarray(inputs["Wk"], np.float32).astype(ml_dtypes.bfloat16)
    Wv = np.asarray(inputs["Wv"], np.float32).astype(ml_dtypes.bfloat16)
    Wo = np.asarray(inputs["Wo"], np.float32).astype(ml_dtypes.bfloat16)

    nblk = tkv // 128
    in_maps = []
    for core in range(8):
        b, hh = divmod(core, 2)
        ch = slice(hh * C, (hh + 1) * C)
        hTb = np.ascontiguousarray(hidden[b].T).astype(ml_dtypes.bfloat16)
        idx = np.nonzero(mask[b])[0]
        n = len(idx)
        assert n <= tkv, f"unmasked keys {n} > TKV {tkv}"
        hTkv = np.zeros((D, tkv), ml_dtypes.bfloat16)
        hTkv[:, :n] = hidden[b].T[:, idx].astype(ml_dtypes.bfloat16)
        mbv = np.full(tkv, NEG, np.float32)
        mbv[:n] = 0.0
        mbv = np.ascontiguousarray(mbv.reshape(nblk, 128).T)  # [128, nblk]
        in_maps.append({
            "hT": hTb,
            "hTkv": hTkv,
            "wq": np.ascontiguousarray(Wq[:, ch]),
            "wk": np.ascontiguousarray(Wk[:, ch]),
            "wv": np.ascontiguousarray(Wv[:, ch]),
            "wo": np.ascontiguousarray(Wo[ch, :]),
            "mb": mbv,
        })
    return in_maps


def gather(results, inputs):
    bo = np.asarray(inputs["bo"], np.float32)
    out = np.empty((B, T, D), np.float32)
    for b in range(B):
        out[b] = results[2 * b]["out"] + results[2 * b + 1]["out"] + bo
    return out


def _pick_tkv(inputs):
    mask = np.asarray(inputs["attention_mask"])
    nmax = int(mask.sum(axis=1).max())
    return max(TKV, -(-nmax // 128) * 128)


def _run(inputs, trace=False):
    tkv = _pick_tkv(inputs)
    nc = _get_nc(tkv)
    in_maps = make_in_maps(inputs, tkv)
    res = run_bass_kernel_spmd(nc, in_maps, core_ids=list(range(8)), trace=trace)
    return gather(res.results, inputs), res


def kernel(**inputs):
    out, _ = _run(inputs)
    return out


# revision 14
# speedup vs baseline: 1.1340x; 1.0741x over previous
"""Multi-head attention kernel for Trainium2, 8 NeuronCores.

Problem: B=4, T=2048, D=1024, 16 heads x 64 head-dim, key-padding mask.
Sharding: core = (batch b, head-half hh); each of the 8 cores computes one
batch's attention over 8 heads (512 channels) and a partial output
projection; the host sums the two partials per batch and adds the bias.

Device-side layout is channel-major throughout:
  hT   = hidden[b]^T                      [D, T]    (bf16)
  Q^T  = (Wq_half)^T hT                   [C, T]    via lhsT=Wq, rhs=hT
  K^T/V computed only at unmasked key positions (host compacts keys using
  the attention mask, zero-padded to TKV).
  S^T  = K^T-block^T Q^T                  [128 keys, 1024] PSUM; both heads
         of a pair side by side (A cols 0:512, B cols 512:1024) for one
         512-query chunk, so a single ACT exp covers the pair.
  P^T  = exp(scale*S^T + bias[k])         (ACT, bias=-1e9 at padded keys)
  O    = [V_h | ones]^T P_h^T             M=65 matmul: rows 0:64 = head
         output, row 64 = softmax denominator (no separate rowsum matmuls).
  norm: PE replicate-matmul broadcasts the denominator row to 64
         partitions, DVE reciprocal_approx_fast, Pool multiply -> ocatT.
  out  = ocatT^T @ Wo_half                [T, D]    (partial, fp32)
"""

import numpy as np
import ml_dtypes

import concourse.bacc as bacc
import concourse.tile as tile
from concourse import mybir
from concourse.bass_utils import run_bass_kernel_spmd

BF16 = mybir.dt.bfloat16
F32 = mybir.dt.float32

B, T, D = 4, 2048, 1024
HEADS, DH = 16, 64
C = 512              # channels per core (8 heads)
SCALE = DH ** -0.5
NEG = -1.0e9

TKV = 1152           # compacted+padded key count (seed-0 max unmasked is 1075)
KB = D // 128        # 8 contraction blocks for the projections
NCB = C // 128       # 4 head-pair blocks
NQC = T // 512       # 4 query chunks


def build_bass(tkv=TKV):
    nblk = tkv // 128
    nc = bacc.Bacc("TRN2", target_bir_lowering=False)

    hT_d = nc.dram_tensor("hT", [D, T], BF16, kind="ExternalInput")
    hTkv_d = nc.dram_tensor("hTkv", [D, tkv], BF16, kind="ExternalInput")
    wq_d = nc.dram_tensor("wq", [D, C], BF16, kind="ExternalInput")
    wk_d = nc.dram_tensor("wk", [D, C], BF16, kind="ExternalInput")
    wv_d = nc.dram_tensor("wv", [D, C], BF16, kind="ExternalInput")
    wo_d = nc.dram_tensor("wo", [C, D], BF16, kind="ExternalInput")
    mb_d = nc.dram_tensor("mb", [128, nblk], F32, kind="ExternalInput")
    out_d = nc.dram_tensor("out", [T, D], F32, kind="ExternalOutput")

    with tile.TileContext(nc) as tc:
        with (
            tc.tile_pool(name="persist", bufs=1) as persist,
            tc.tile_pool(name="pt", bufs=3) as ppool,
            tc.tile_pool(name="osb", bufs=4) as osbp,
            tc.tile_pool(name="rsbp", bufs=4) as rsbp,
            tc.tile_pool(name="ostage", bufs=4) as ostage,
        ):
            # ---- persistent SBUF tensors ----
            hT = persist.tile([128, KB, T], BF16)
            hTkv = persist.tile([128, KB, tkv], BF16)
            wq = persist.tile([128, KB, C], BF16)
            wk = persist.tile([128, KB, C], BF16)
            wv = persist.tile([128, KB, C], BF16)
            wo = persist.tile([128, NCB, D], BF16)
            mb = persist.tile([128, nblk], F32)
            qT = persist.tile([128, NCB, T], BF16)
            kT = persist.tile([128, NCB, tkv], BF16)
            vext = persist.tile([128, nblk, 8, 65], BF16)
            ocatT = persist.tile([128, NCB, T], BF16)
            ones32 = persist.tile([128, 64], F32)
            warm = persist.tile([128, 8], F32)

            # constants first: no DMA deps, lets the exp table preload run
            # at t=0 (first Exp ACT triggers the ~2.7us table-set load)
            nc.vector.memset(ones32[:], 1.0)
            nc.vector.memset(vext[:, :, :, 64], 1.0)
            nc.scalar.activation(
                warm[:], ones32[:, 0:8], mybir.ActivationFunctionType.Exp,
            )

            # ---- input DMA, in dependency-priority order ----
            hTkv_r = hTkv_d.ap().rearrange("(k p) t -> p k t", p=128)
            hT_r = hT_d.ap().rearrange("(k p) t -> p k t", p=128)
            nc.sync.dma_start(wk[:], wk_d.ap().rearrange("(k p) c -> p k c", p=128))
            for k in range(KB):
                nc.sync.dma_start(hTkv[:, k, :], hTkv_r[:, k, :])
            nc.sync.dma_start(wq[:], wq_d.ap().rearrange("(k p) c -> p k c", p=128))
            # queries chunk-major so Q(cb=0, qc=0/1) unblocks early
            for tch in range(NQC):
                for k in range(KB):
                    nc.sync.dma_start(
                        hT[:, k, tch * 512:(tch + 1) * 512],
                        hT_r[:, k, tch * 512:(tch + 1) * 512],
                    )
            nc.gpsimd.dma_start(wv[:], wv_d.ap().rearrange("(k p) c -> p k c", p=128))
            nc.gpsimd.dma_start(mb[:], mb_d.ap())
            nc.gpsimd.dma_start(wo[:], wo_d.ap().rearrange("(k p) e -> p k e", p=128))

            # ---- projections ----
            with tc.tile_pool(name="qkv_ps", bufs=2, space="PSUM") as qps:
                def emit_v(tb):
                    ps = qps.tile([128, 512], F32, tag="qkv", name=f"vps_{tb}")
                    for k in range(KB):
                        nc.tensor.matmul(
                            ps[:],
                            hTkv[:, k, tb * 128:(tb + 1) * 128],
                            wv[:, k, :],
                            start=(k == 0), stop=(k == KB - 1),
                        )
                    # scatter the 8 heads' 64-wide slices into vext (ones col
                    # at 64 stays from the memset)
                    nc.vector.tensor_copy(vext[:, tb, :, 0:64], ps[:])

                def emit_k(cb):
                    for t0 in range(0, tkv, 512):
                        w = min(512, tkv - t0)
                        ps = qps.tile([128, 512], F32, tag="qkv", name=f"kps_{cb}_{t0}")
                        for k in range(KB):
                            nc.tensor.matmul(
                                ps[:, :w],
                                wk[:, k, cb * 128:(cb + 1) * 128],
                                hTkv[:, k, t0:t0 + w],
                                start=(k == 0), stop=(k == KB - 1),
                            )
                        nc.vector.tensor_copy(kT[:, cb, t0:t0 + w], ps[:, :w])

                def emit_q(cb, tchs):
                    for tch in tchs:
                        ps = qps.tile([128, 512], F32, tag="qkv", name=f"qps_{cb}_{tch}")
                        for k in range(KB):
                            nc.tensor.matmul(
                                ps[:],
                                wq[:, k, cb * 128:(cb + 1) * 128],
                                hT[:, k, tch * 512:(tch + 1) * 512],
                                start=(k == 0), stop=(k == KB - 1),
                            )
                        nc.vector.tensor_copy(qT[:, cb, tch * 512:(tch + 1) * 512], ps[:])

                def gen_k(cb):
                    # one PE matmul per yield; eviction rides the last one
                    for t0 in range(0, tkv, 512):
                        w = min(512, tkv - t0)
                        ps = qps.tile([128, 512], F32, tag="qkv",
                                      name=f"kps_{cb}_{t0}", uniquify=True)
                        for k in range(KB):
                            nc.tensor.matmul(
                                ps[:, :w],
                                wk[:, k, cb * 128:(cb + 1) * 128],
                                hTkv[:, k, t0:t0 + w],
                                start=(k == 0), stop=(k == KB - 1),
                            )
                            if k == KB - 1:
                                nc.vector.tensor_copy(kT[:, cb, t0:t0 + w], ps[:, :w])
                            yield

                def gen_q(cb, tch):
                    ps = qps.tile([128, 512], F32, tag="qkv",
                                  name=f"qps_{cb}_{tch}", uniquify=True)
                    for k in range(KB):
                        nc.tensor.matmul(
                            ps[:],
                            wq[:, k, cb * 128:(cb + 1) * 128],
                            hT[:, k, tch * 512:(tch + 1) * 512],
                            start=(k == 0), stop=(k == KB - 1),
                        )
                        if k == KB - 1:
                            nc.vector.tensor_copy(
                                qT[:, cb, tch * 512:(tch + 1) * 512], ps[:])
                        yield

                # ---- attention ----
                with (
                    tc.tile_pool(name="s_ps", bufs=2, space="PSUM") as spool,
                    tc.tile_pool(name="o_ps", bufs=2, space="PSUM") as opool,
                ):
                    # ---- filler machinery: one PE matmul per pump ----
                    fill_q = []

                    def pump_fillers(n):
                        while n > 0 and fill_q:
                            try:
                                next(fill_q[0])
                                n -= 1
                            except StopIteration:
                                fill_q.pop(0)

                    def gen_outproj(tt):
                        ot = ostage.tile([128, 1024], F32, tag="os",
                                         name=f"ot_{tt}", uniquify=True)
                        for e in range(2):
                            ps = opool.tile([128, 512], F32, tag="o",
                                            name=f"pj_{tt}_{e}", uniquify=True)
                            for cbj in range(NCB):
                                nc.tensor.matmul(
                                    ps[:],
                                    ocatT[:, cbj, tt * 128:(tt + 1) * 128],
                                    wo[:, cbj, e * 512:(e + 1) * 512],
                                    start=(cbj == 0), stop=(cbj == NCB - 1),
                                )
                                if cbj == NCB - 1:
                                    nc.vector.tensor_copy(
                                        ot[:, e * 512:(e + 1) * 512], ps[:])
                                    if e == 1:
                                        nc.sync.dma_start(
                                            out_d.ap()[tt * 128:(tt + 1) * 128, :],
                                            ot[:],
                                        )
                                yield

                    def emit_normalize(cb, qc, oA, oB):
                        qs = slice(qc * 512, (qc + 1) * 512)
                        for h, op in ((0, oA), (1, oB)):
                            o_sb = osbp.tile([65, 512], F32, tag="ob",
                                             name=f"osb_{cb}_{qc}_{h}")
                            nc.vector.tensor_copy(o_sb[:], op[0:65, :])
                            repl = opool.tile([64, 512], F32, tag="o",
                                              name=f"rp_{cb}_{qc}_{h}")
                            nc.tensor.matmul(
                                repl[:], ones32[64:65, :], o_sb[64:65, :],
                                start=True, stop=True,
                            )
                            rsb = rsbp.tile([64, 512], F32, tag="rs",
                                            name=f"rsb_{cb}_{qc}_{h}")
                            nc.vector.reciprocal_approx_fast(rsb[:], repl[:])
                            rows = slice(h * 64, (h + 1) * 64)
                            nc.gpsimd.tensor_mul(
                                ocatT[rows, cb, qs], o_sb[0:64, :], rsb[:],
                            )

                    # ---- prelude: V, K(0), Q(0, tch0) on the PE ----
                    emit_k(0)
                    emit_q(0, [0])
                    for tb in range(nblk):
                        emit_v(tb)

                    # filler queue: remaining projections, earliest deadline
                    # first (cb-outer attention order below)
                    fill_q.append(gen_q(0, 1))
                    fill_q.append(gen_q(0, 2))
                    fill_q.append(gen_q(0, 3))
                    for cbn in (1, 2, 3):
                        fill_q.append(gen_k(cbn))
                        for tch in range(NQC):
                            fill_q.append(gen_q(cbn, tch))

                    # ---- attention: software-pipelined PE stream ----
                    # per sub-iter: S(k) -> EXP(k) -> [O(k-1) deferred] ->
                    # fillers; O(k) lands in the next sub-iter's EXP shadow.
                    pend = None  # (pt, cb, qc, blk)
                    o_tiles = {}

                    def deferred_o(ppt, pcb, pqc, pblk):
                        # O accumulator tiles allocate lazily here so the pool
                        # rotation matches emission order (normalize(k) frees
                        # slots before (k+1)'s accumulators claim them)
                        if (pcb, pqc) not in o_tiles:
                            o_tiles[(pcb, pqc)] = (
                                opool.tile([128, 512], F32, tag="o",
                                           name=f"oA_{pcb}_{pqc}", uniquify=True),
                                opool.tile([128, 512], F32, tag="o",
                                           name=f"oB_{pcb}_{pqc}", uniquify=True),
                            )
                        poA, poB = o_tiles[(pcb, pqc)]
                        st_ = (pblk == 0)
                        sp_ = (pblk == nblk - 1)
                        nc.tensor.matmul(
                            poA[0:65, :], vext[:, pblk, 2 * pcb, :],
                            ppt[:, 0:512], start=st_, stop=sp_,
                        )
                        nc.tensor.matmul(
                            poB[0:65, :], vext[:, pblk, 2 * pcb + 1, :],
                            ppt[:, 512:1024], start=st_, stop=sp_,
                        )
                        if sp_:
                            del o_tiles[(pcb, pqc)]
                            emit_normalize(pcb, pqc, poA, poB)
                        return sp_

                    for cb in range(NCB):
                        for qc in range(NQC):
                            qs = slice(qc * 512, (qc + 1) * 512)
                            for blk in range(nblk):
                                st = spool.tile([128, 1024], F32, tag="s",
                                                name=f"st_{cb}_{qc}_{blk}")
                                ks = slice(blk * 128, (blk + 1) * 128)
                                nc.tensor.matmul(
                                    st[:, 0:512],
                                    kT[0:64, cb, ks], qT[0:64, cb, qs],
                                    start=True, stop=True,
                                )
                                nc.tensor.matmul(
                                    st[:, 512:1024],
                                    kT[64:128, cb, ks], qT[64:128, cb, qs],
                                    start=True, stop=True,
                                )
                                pt = ppool.tile([128, 1024], BF16, tag="pt")
                                nc.scalar.activation(
                                    pt[:], st[:],
                                    mybir.ActivationFunctionType.Exp,
                                    bias=mb[:, blk:blk + 1], scale=SCALE,
                                )
                                if pend is not None:
                                    if not deferred_o(*pend):
                                        pump_fillers(2)
                                else:
                                    pump_fillers(2)
                                pend = (pt, cb, qc, blk)
                            if cb == NCB - 1:
                                # this qc's ocatT completes once pend's O+norm
                                # lands; queue its output projection
                                for tt in range(4 * qc, 4 * qc + 4):
                                    fill_q.append(gen_outproj(tt))
                    # drain
                    deferred_o(*pend)
                    pump_fillers(10 ** 9)

    nc.compile()
    return nc


_NC_CACHE = {}


def _get_nc(tkv=TKV):
    if tkv not in _NC_CACHE:
        _NC_CACHE[tkv] = build_bass(tkv)
    return _NC_CACHE[tkv]


def make_in_maps(inputs, tkv=TKV):
    hidden = np.asarray(inputs["hidden_states"], np.float32)
    mask = np.asarray(inputs["attention_mask"])
    Wq = np.asarray(inputs["Wq"], np.float32).astype(ml_dtypes.bfloat16)
    Wk = np.asarray(inputs["Wk"], np.float32).astype(ml_dtypes.bfloat16)
    Wv = np.asarray(inputs["Wv"], np.float32).astype(ml_dtypes.bfloat16)
    Wo = np.asarray(inputs["Wo"], np.float32).astype(ml_dtypes.bfloat16)

    nblk = tkv // 128
    in_maps = []
    for core in range(8):
        b, hh = divmod(core, 2)
        ch = slice(hh * C, (hh + 1) * C)
        hTb = np.ascontiguousarray(hidden[b].T).astype(ml_dtypes.bfloat16)
        idx = np.nonzero(mask[b])[0]
        n = len(idx)
        assert n <= tkv, f"unmasked keys {n} > TKV {tkv}"
        hTkv = np.zeros((D, tkv), ml_dtypes.bfloat16)
        hTkv[:, :n] = hidden[b].T[:, idx].astype(ml_dtypes.bfloat16)
        mbv = np.full(tkv, NEG, np.float32)
        mbv[:n] = 0.0
        mbv = np.ascontiguousarray(mbv.reshape(nblk, 128).T)  # [128, nblk]
        in_maps.append({
            "hT": hTb,
            "hTkv": hTkv,
            "wq": np.ascontiguousarray(Wq[:, ch]),
            "wk": np.ascontiguousarray(Wk[:, ch]),
            "wv": np.ascontiguousarray(Wv[:, ch]),
            "wo": np.ascontiguousarray(Wo[ch, :]),
            "mb": mbv,
        })
    return in_maps


def gather(results, inputs):
    bo = np.asarray(inputs["bo"], np.float32)
    out = np.empty((B, T, D), np.float32)
    for b in range(B):
        out[b] = results[2 * b]["out"] + results[2 * b + 1]["out"] + bo
    return out


def _pick_tkv(inputs):
    mask = np.asarray(inputs["attention_mask"])
    nmax = int(mask.sum(axis=1).max())
    return max(TKV, -(-nmax // 128) * 128)


def _run(inputs, trace=False):
    tkv = _pick_tkv(inputs)
    nc = _get_nc(tkv)
    in_maps = make_in_maps(inputs, tkv)
    res = run_bass_kernel_spmd(nc, in_maps, core_ids=list(range(8)), trace=trace)
    return gather(res.results, inputs), res


def kernel(**inputs):
    out, _ = _run(inputs)
    return out


# revision 24
# speedup vs baseline: 1.3419x; 1.1834x over previous
"""Multi-head attention kernel for Trainium2, 8 NeuronCores.

Problem: B=4, T=2048, D=1024, 16 heads x 64 head-dim, key-padding mask.
Sharding: core = (batch b, head-half hh); each of the 8 cores computes one
batch's attention over 8 heads (512 channels) and a partial output
projection; the host sums the two partials per batch and adds the bias.

Device-side layout is channel-major throughout:
  hT   = hidden[b]^T                      [D, T]    (bf16)
  Q^T  = (Wq_half)^T hT                   [C, T]    via lhsT=Wq, rhs=hT
  K^T/V computed only at unmasked key positions (host compacts keys using
  the attention mask, zero-padded to TKV).
  S^T  = K^T-block^T Q^T                  [128 keys, 1024] PSUM; both heads
         of a pair side by side (A cols 0:512, B cols 512:1024) for one
         512-query chunk, so a single ACT exp covers the pair.
  P^T  = exp(scale*S^T + bias[k])         (ACT, bias=-1e9 at padded keys)
  O    = [V_h | ones]^T P_h^T             M=65 matmul: rows 0:64 = head
         output, row 64 = softmax denominator (no separate rowsum matmuls).
  norm: PE replicate-matmul broadcasts the denominator row to 64
         partitions, DVE reciprocal_approx_fast, Pool multiply -> ocatT.
  out  = ocatT^T @ Wo_half                [T, D]    (partial, fp32)
"""

import numpy as np
import ml_dtypes

import concourse.bacc as bacc
import concourse.tile as tile
from concourse import mybir
from concourse.bass_utils import run_bass_kernel_spmd

BF16 = mybir.dt.bfloat16
F32 = mybir.dt.float32

B, T, D = 4, 2048, 1024
HEADS, DH = 16, 64
C = 512              # channels per core (8 heads)
SCALE = DH ** -0.5
NEG = -1.0e9

TKV = 1152           # compacted+padded key count (seed-0 max unmasked is 1075)
KB = D // 128        # 8 contraction blocks for the projections
NCB = C // 128       # 4 head-pair blocks
NQC = T // 512       # 4 query chunks


def build_bass(tkv=TKV):
    nblk = tkv // 128
    nc = bacc.Bacc("TRN2", target_bir_lowering=False)

    hT_d = nc.dram_tensor("hT", [D, T], BF16, kind="ExternalInput")
    hTkv_d = nc.dram_tensor("hTkv", [D, tkv], BF16, kind="ExternalInput")
    wq_d = nc.dram_tensor("wq", [D, C], BF16, kind="ExternalInput")
    wk_d = nc.dram_tensor("wk", [D, C], BF16, kind="ExternalInput")
    wv_d = nc.dram_tensor("wv", [D, C], BF16, kind="ExternalInput")
    wo_d = nc.dram_tensor("wo", [C, D], BF16, kind="ExternalInput")
    mb_d = nc.dram_tensor("mb", [128, nblk], F32, kind="ExternalInput")
    out_d = nc.dram_tensor("out", [T, D], F32, kind="ExternalOutput")

    with tile.TileContext(nc) as tc:
        with (
            tc.tile_pool(name="persist", bufs=1) as persist,
            tc.tile_pool(name="pt", bufs=3) as ppool,
            tc.tile_pool(name="osb", bufs=4) as osbp,
            tc.tile_pool(name="rsbp", bufs=4) as rsbp,
            tc.tile_pool(name="ostage", bufs=4) as ostage,
        ):
            # ---- persistent SBUF tensors ----
            hT = persist.tile([128, KB, T], BF16)
            hTkv = persist.tile([128, KB, tkv], BF16)
            wq = persist.tile([128, KB, C], BF16)
            wk = persist.tile([128, KB, C], BF16)
            wv = persist.tile([128, KB, C], BF16)
            wo = persist.tile([128, NCB, D], BF16)
            mb = persist.tile([128, nblk], F32)
            qT = persist.tile([128, NCB, T], BF16)
            kT = persist.tile([128, NCB, tkv], BF16)
            # per (blk, head): [V_h (64 cols) | ones (64 cols)] so one M=128
            # O-matmul yields the head output in rows 0:64 AND the softmax
            # denominator replicated across rows 64:128
            vext = persist.tile([128, nblk * 8, 128], BF16)
            ocatT = persist.tile([128, NCB, T], BF16)
            warm = persist.tile([128, 8], F32)

            # constants first: no DMA deps, lets the exp table preload run
            # at t=0 (first Exp ACT triggers the ~2.7us table-set load)
            nc.vector.memset(vext[:, :, 64:128], 1.0)
            nc.scalar.activation(
                warm[:], vext[0:128, 0:1, 64:72], mybir.ActivationFunctionType.Exp,
            )

            # ---- input DMA, in dependency-priority order ----
            hTkv_r = hTkv_d.ap().rearrange("(k p) t -> p k t", p=128)
            hT_r = hT_d.ap().rearrange("(k p) t -> p k t", p=128)
            nc.sync.dma_start(wk[:], wk_d.ap().rearrange("(k p) c -> p k c", p=128))
            for k in range(KB):
                nc.sync.dma_start(hTkv[:, k, :], hTkv_r[:, k, :])
            nc.sync.dma_start(wq[:], wq_d.ap().rearrange("(k p) c -> p k c", p=128))
            # queries chunk-major so Q(cb=0, qc=0/1) unblocks early
            for tch in range(NQC):
                for k in range(KB):
                    nc.sync.dma_start(
                        hT[:, k, tch * 512:(tch + 1) * 512],
                        hT_r[:, k, tch * 512:(tch + 1) * 512],
                    )
            nc.gpsimd.dma_start(wv[:], wv_d.ap().rearrange("(k p) c -> p k c", p=128))
            nc.gpsimd.dma_start(mb[:], mb_d.ap())
            nc.gpsimd.dma_start(wo[:], wo_d.ap().rearrange("(k p) e -> p k e", p=128))

            # ---- projections ----
            with tc.tile_pool(name="qkv_ps", bufs=2, space="PSUM") as qps:
                def emit_v(tb):
                    ps = qps.tile([128, 512], F32, tag="qkv", name=f"vps_{tb}")
                    for k in range(KB):
                        nc.tensor.matmul(
                            ps[:],
                            hTkv[:, k, tb * 128:(tb + 1) * 128],
                            wv[:, k, :],
                            start=(k == 0), stop=(k == KB - 1),
                        )
                    # scatter the 8 heads' 64-wide slices into vext (ones
                    # cols 64:128 stay from the memset)
                    nc.vector.tensor_copy(vext[:, tb * 8:(tb + 1) * 8, 0:64], ps[:])

                def emit_k(cb):
                    for t0 in range(0, tkv, 512):
                        w = min(512, tkv - t0)
                        ps = qps.tile([128, 512], F32, tag="qkv", name=f"kps_{cb}_{t0}")
                        for k in range(KB):
                            nc.tensor.matmul(
                                ps[:, :w],
                                wk[:, k, cb * 128:(cb + 1) * 128],
                                hTkv[:, k, t0:t0 + w],
                                start=(k == 0), stop=(k == KB - 1),
                            )
                        nc.vector.tensor_copy(kT[:, cb, t0:t0 + w], ps[:, :w])

                def emit_q(cb, tchs):
                    for tch in tchs:
                        ps = qps.tile([128, 512], F32, tag="qkv", name=f"qps_{cb}_{tch}")
                        for k in range(KB):
                            nc.tensor.matmul(
                                ps[:],
                                wq[:, k, cb * 128:(cb + 1) * 128],
                                hT[:, k, tch * 512:(tch + 1) * 512],
                                start=(k == 0), stop=(k == KB - 1),
                            )
                        nc.vector.tensor_copy(qT[:, cb, tch * 512:(tch + 1) * 512], ps[:])

                def gen_k(cb):
                    # one PE matmul per yield; eviction rides the last one
                    for t0 in range(0, tkv, 512):
                        w = min(512, tkv - t0)
                        ps = qps.tile([128, 512], F32, tag="qkv",
                                      name=f"kps_{cb}_{t0}", uniquify=True)
                        for k in range(KB):
                            nc.tensor.matmul(
                                ps[:, :w],
                                wk[:, k, cb * 128:(cb + 1) * 128],
                                hTkv[:, k, t0:t0 + w],
                                start=(k == 0), stop=(k == KB - 1),
                            )
                            if k == KB - 1:
                                nc.vector.tensor_copy(kT[:, cb, t0:t0 + w], ps[:, :w])
                            yield

                def gen_q(cb, tch):
                    ps = qps.tile([128, 512], F32, tag="qkv",
                                  name=f"qps_{cb}_{tch}", uniquify=True)
                    for k in range(KB):
                        nc.tensor.matmul(
                            ps[:],
                            wq[:, k, cb * 128:(cb + 1) * 128],
                            hT[:, k, tch * 512:(tch + 1) * 512],
                            start=(k == 0), stop=(k == KB - 1),
                        )
                        if k == KB - 1:
                            nc.vector.tensor_copy(
                                qT[:, cb, tch * 512:(tch + 1) * 512], ps[:])
                        yield

                # ---- attention ----
                with (
                    tc.tile_pool(name="s_ps", bufs=2, space="PSUM") as spool,
                    tc.tile_pool(name="o_ps", bufs=2, space="PSUM") as opool,
                ):
                    # ---- filler machinery: one PE matmul per pump ----
                    fill_q = []

                    def pump_fillers(n):
                        while n > 0 and fill_q:
                            try:
                                next(fill_q[0])
                                n -= 1
                            except StopIteration:
                                fill_q.pop(0)

                    def gen_outproj(tt):
                        ot = ostage.tile([128, 1024], F32, tag="os",
                                         name=f"ot_{tt}", uniquify=True)
                        for e in range(2):
                            ps = opool.tile([128, 512], F32, tag="o",
                                            name=f"pj_{tt}_{e}", uniquify=True)
                            for cbj in range(NCB):
                                nc.tensor.matmul(
                                    ps[:],
                                    ocatT[:, cbj, tt * 128:(tt + 1) * 128],
                                    wo[:, cbj, e * 512:(e + 1) * 512],
                                    start=(cbj == 0), stop=(cbj == NCB - 1),
                                )
                                if cbj == NCB - 1:
                                    nc.vector.tensor_copy(
                                        ot[:, e * 512:(e + 1) * 512], ps[:])
                                    if e == 1:
                                        nc.sync.dma_start(
                                            out_d.ap()[tt * 128:(tt + 1) * 128, :],
                                            ot[:],
                                        )
                                yield

                    def emit_normalize(cb, qc, oA, oB):
                        # O rows 0:64, replicated denominator rows 64:128.
                        # DVE ops stay partition-aligned; the GpSimd multiply
                        # handles the partition shift into ocatT.
                        qs = slice(qc * 512, (qc + 1) * 512)
                        for h, op in ((0, oA), (1, oB)):
                            o_sb = osbp.tile([128, 512], F32, tag="ob",
                                             name=f"osb_{cb}_{qc}_{h}")
                            nc.vector.tensor_copy(o_sb[:], op[:, :])
                            # DMA-shift the denominator rows to partition 0:
                            # reciprocal_approx_fast only works at base 0, and
                            # engines cannot shift partitions themselves
                            dsb = rsbp.tile([64, 512], F32, tag="rh",
                                            name=f"dsb_{cb}_{qc}_{h}")
                            nc.gpsimd.dma_start(dsb[:], o_sb[64:128, :])
                            rsb = rsbp.tile([64, 512], F32, tag="rs",
                                            name=f"rsb_{cb}_{qc}_{h}")
                            nc.vector.reciprocal_approx_fast(rsb[:], dsb[:])
                            rows = slice(h * 64, (h + 1) * 64)
                            nc.gpsimd.tensor_mul(
                                ocatT[rows, cb, qs], o_sb[0:64, :], rsb[:],
                            )

                    # ---- prelude: V, K(0), Q(0, tch0) on the PE ----
                    emit_k(0)
                    emit_q(0, [0])
                    for tb in range(nblk):
                        emit_v(tb)

                    # filler queue: remaining projections, earliest deadline
                    # first (cb-outer attention order below)
                    fill_q.append(gen_q(0, 1))
                    fill_q.append(gen_q(0, 2))
                    fill_q.append(gen_q(0, 3))
                    for cbn in (1, 2, 3):
                        fill_q.append(gen_k(cbn))
                        for tch in range(NQC):
                            fill_q.append(gen_q(cbn, tch))

                    # ---- attention: software-pipelined PE stream ----
                    # per sub-iter: S(k) -> EXP(k) -> [O(k-1) deferred] ->
                    # fillers; O(k) lands in the next sub-iter's EXP shadow.
                    pend = None  # (pt, cb, qc, blk)
                    o_tiles = {}

                    def deferred_o(ppt, pcb, pqc, pblk):
                        # O accumulator tiles allocate lazily here so the pool
                        # rotation matches emission order (normalize(k) frees
                        # slots before (k+1)'s accumulators claim them)
                        if (pcb, pqc) not in o_tiles:
                            o_tiles[(pcb, pqc)] = (
                                opool.tile([128, 512], F32, tag="o",
                                           name=f"oA_{pcb}_{pqc}", uniquify=True),
                                opool.tile([128, 512], F32, tag="o",
                                           name=f"oB_{pcb}_{pqc}", uniquify=True),
                            )
                        poA, poB = o_tiles[(pcb, pqc)]
                        st_ = (pblk == 0)
                        sp_ = (pblk == nblk - 1)
                        nc.tensor.matmul(
                            poA[:, :], vext[:, pblk * 8 + 2 * pcb, :],
                            ppt[:, 0:512], start=st_, stop=sp_,
                        )
                        nc.tensor.matmul(
                            poB[:, :], vext[:, pblk * 8 + 2 * pcb + 1, :],
                            ppt[:, 512:1024], start=st_, stop=sp_,
                        )
                        if sp_:
                            del o_tiles[(pcb, pqc)]
                            emit_normalize(pcb, pqc, poA, poB)
                        return sp_

                    for cb in range(NCB):
                        for qc in range(NQC):
                            qs = slice(qc * 512, (qc + 1) * 512)
                            for blk in range(nblk):
                                st = spool.tile([128, 1024], F32, tag="s",
                                                name=f"st_{cb}_{qc}_{blk}")
                                ks = slice(blk * 128, (blk + 1) * 128)
                                nc.tensor.matmul(
                                    st[:, 0:512],
                                    kT[0:64, cb, ks], qT[0:64, cb, qs],
                                    start=True, stop=True,
                                )
                                nc.tensor.matmul(
                                    st[:, 512:1024],
                                    kT[64:128, cb, ks], qT[64:128, cb, qs],
                                    start=True, stop=True,
                                )
                                pt = ppool.tile([128, 1024], BF16, tag="pt")
                                nc.scalar.activation(
                                    pt[:], st[:],
                                    mybir.ActivationFunctionType.Exp,
                                    bias=mb[:, blk:blk + 1], scale=SCALE,
                                )
                                if pend is not None:
                                    if not deferred_o(*pend):
                                        pump_fillers(2)
                                else:
                                    pump_fillers(2)
                                pend = (pt, cb, qc, blk)
                            if cb == NCB - 1:
                                # this qc's ocatT completes once pend's O+norm
                                # lands; queue its output projection
                                for tt in range(4 * qc, 4 * qc + 4):
                                    fill_q.append(gen_outproj(tt))
                    # drain
                    deferred_o(*pend)
                    pump_fillers(10 ** 9)

    nc.compile()
    return nc


_NC_CACHE = {}


def _get_nc(tkv=TKV):
    if tkv not in _NC_CACHE:
        _NC_CACHE[tkv] = build_bass(tkv)
    return _NC_CACHE[tkv]


def make_in_maps(inputs, tkv=TKV):
    hidden = np.asarray(inputs["hidden_states"], np.float32)
    mask = np.asarray(inputs["attention_mask"])
    Wq = np.asarray(inputs["Wq"], np.float32).astype(ml_dtypes.bfloat16)
    Wk = np.asarray(inputs["Wk"], np.float32).astype(ml_dtypes.bfloat16)
    Wv = np.asarray(inputs["Wv"], np.float32).astype(ml_dtypes.bfloat16)
    Wo = np.asarray(inputs["Wo"], np.float32).astype(ml_dtypes.bfloat16)

    nblk = tkv // 128
    in_maps = []
    for core in range(8):
        b, hh = divmod(core, 2)
        ch = slice(hh * C, (hh + 1) * C)
        hTb = np.ascontiguousarray(hidden[b].T).astype(ml_dtypes.bfloat16)
        idx = np.nonzero(mask[b])[0]
        n = len(idx)
        assert n <= tkv, f"unmasked keys {n} > TKV {tkv}"
        hTkv = np.zeros((D, tkv), ml_dtypes.bfloat16)
        hTkv[:, :n] = hidden[b].T[:, idx].astype(ml_dtypes.bfloat16)
        mbv = np.full(tkv, NEG, np.float32)
        mbv[:n] = 0.0
        mbv = np.ascontiguousarray(mbv.reshape(nblk, 128).T)  # [128, nblk]
        in_maps.append({
            "hT": hTb,
            "hTkv": hTkv,
            "wq": np.ascontiguousarray(Wq[:, ch]),
            "wk": np.ascontiguousarray(Wk[:, ch]),
            "wv": np.ascontiguousarray(Wv[:, ch]),
            "wo": np.ascontiguousarray(Wo[ch, :]),
            "mb": mbv,
        })
    return in_maps


def gather(results, inputs):
    bo = np.asarray(inputs["bo"], np.float32)
    out = np.empty((B, T, D), np.float32)
    for b in range(B):
        out[b] = results[2 * b]["out"] + results[2 * b + 1]["out"] + bo
    return out


def _pick_tkv(inputs):
    mask = np.asarray(inputs["attention_mask"])
    nmax = int(mask.sum(axis=1).max())
    return max(TKV, -(-nmax // 128) * 128)


def _run(inputs, trace=False):
    tkv = _pick_tkv(inputs)
    nc = _get_nc(tkv)
    in_maps = make_in_maps(inputs, tkv)
    res = run_bass_kernel_spmd(nc, in_maps, core_ids=list(range(8)), trace=trace)
    return gather(res.results, inputs), res


def kernel(**inputs):
    out, _ = _run(inputs)
    return out
